# revision 27
# baseline (speedup 1.0000x reference)
"""DecoderTreeRNN Trainium2 kernel.

Computes: h0 = relu(encoding); expand a depth-`depth` binary tree with two
zero-input GRU cells (left/right); project every leaf hidden state with W_out
and take log_softmax over the vocab.

Strategy: pure data parallel over 8 NeuronCores (batch sharded), GRU weights
and the output projection replicated.  On-core layout is transposed
([hidden-chunk on partitions, tokens on the free dim]) so all matmuls
contract over partitions and the softmax reduction runs along the free dim.

v2: the output projection runs in fp8 (DoubleRow matmuls, 2x tensor rate),
y and the stored output are bf16 (host upcasts to fp32; log-probs ~-9 so
bf16 keeps elementwise rel err ~2e-3, well under the 2e-2 gate), and the
log_softmax tail subtract is split DVE/ACT per segment.
"""

import os
import sys
from contextlib import ExitStack

import numpy as np

for _p in ("/opt/trn_rl_repo", "/root/.axon_site/_ro/trn_rl_repo"):
    if os.path.isdir(_p) and _p not in sys.path:
        sys.path.insert(0, _p)

import ml_dtypes

N_CORES = 8
P = 128
TTILE = 512  # token tile for GRU matmuls (max fp32 moving free dim)
NBF = 512  # fp32 elements per PSUM bank
VGW = 4 * NBF  # vocab group width (4 PSUM banks; 2 rotating slots)
EGW = VGW  # exp/accumulate granularity (per drain group: exp starts sooner)
# final-subtract engine per vocab segment within a chunk: V=DVE tensor_scalar
# (4x mode on bf16), A=ACT Identity+bias, G=GPSIMD tensor_scalar (ACT is the
# saturated engine in the projection; GPSIMD is otherwise idle)
SEG_ENGINES = ("V", "G", "V", "G", "V")
# output DMA issued after these vocab segments (batched; covers back to the
# previous flush point)
DMA_AFTER = (1, 3, 4)

# Set by test harness to capture a profile on the next kernel() call.
TRACE = False
LAST_EXEC_NS = None
LAST_RESULTS = None

_COMPILE_CACHE = {}


def _bitrev(x, bits):
    r = 0
    for _ in range(bits):
        r = (r << 1) | (x & 1)
        x >>= 1
    return r


def _numpy_reference(encoding, W_hh_l, b_ih_l, b_hh_l, W_hh_r, b_ih_r, b_hh_r,
                     W_out, b_out, depth):
    def gru(h, W, b_ih, b_hh):
        Hd = h.shape[-1]
        gh = h @ W.T + b_hh
        r = 1.0 / (1.0 + np.exp(-(b_ih[:Hd] + gh[..., :Hd])))
        z = 1.0 / (1.0 + np.exp(-(b_ih[Hd:2 * Hd] + gh[..., Hd:2 * Hd])))
        n = np.tanh(b_ih[2 * Hd:] + r * gh[..., 2 * Hd:])
        return (1.0 - z) * n + z * h

    h = np.maximum(encoding, 0.0)[:, None, :]
    for _ in range(depth):
        left = gru(h, W_hh_l, b_ih_l, b_hh_l)
        right = gru(h, W_hh_r, b_ih_r, b_hh_r)
        h = np.stack([left, right], axis=2).reshape(h.shape[0], -1, h.shape[-1])
    logits = h @ W_out.T + b_out
    m = logits.max(axis=-1, keepdims=True)
    e = np.exp(logits - m)
    return (logits - m) - np.log(e.sum(axis=-1, keepdims=True))


def _patch_act_tables(bacc, mybir):
    """Constrain the ACT table-set chooser so the GRU phase and the
    projection phase each stick to ONE set (2 loads total instead of 2
    per token chunk).  Only the chooser's view is filtered; the runtime
    tables are the real (full) sets, so execution is unchanged."""
    from concourse import hw_specs
    AF = mybir.ActivationFunctionType
    orig = hw_specs.get_activation_tables
    if getattr(bacc.get_activation_tables, "_treernn_patch", False):
        return
    keep = {
        "sigmoid_and_others": {AF.Sigmoid, AF.Tanh, AF.Relu},
        "natural_log_exp_and_others": {AF.Exp, AF.Ln, AF.Identity, AF.Copy},
    }
    controlled = set().union(*keep.values())

    def patched(arch):
        tabs = {k: set(v) for k, v in orig(arch).items()}
        for name, s in tabs.items():
            s -= controlled
            s |= keep.get(name, set())
        return tabs

    patched._treernn_patch = True
    bacc.get_activation_tables = patched


def _build(Bc, H, V, depth):
    """Build + compile the single-core SPMD program (identical on all cores)."""
    import concourse.bass as bass  # noqa: F401
    import concourse.tile as tile
    from concourse import bacc, mybir

    f32 = mybir.dt.float32
    bf16 = mybir.dt.bfloat16
    f8 = mybir.dt.float8e4
    AF = mybir.ActivationFunctionType
    OP = mybir.AluOpType
    DR = mybir.MatmulPerfMode.DoubleRow
    _patch_act_tables(bacc, mybir)

    KH = H // P
    H3 = 3 * H
    L = 1 << depth
    TOK = Bc * L
    NTC = (TOK + P - 1) // P

    def _chunks(width):
        out, pos = [], 0
        while pos < V:
            w = min(width, V - pos)
            out.append((pos, w))
            pos += w
        return out

    vgroups = _chunks(VGW)   # PSUM drain / final-subtract granularity
    NVG = len(vgroups)
    # exp groups end at these vgroup indices (vgroup-aligned, ~EGW wide)
    exp_end_vg = []
    acc = 0
    for vg, (vs, vw) in enumerate(vgroups):
        acc += vw
        if acc >= EGW or vg == NVG - 1:
            exp_end_vg.append(vg)
            acc = 0
    NEG = len(exp_end_vg)

    nc = bacc.Bacc("TRN2", target_bir_lowering=False, debug=False,
                   num_devices=N_CORES)

    enc_d = nc.dram_tensor("enc_t", [P, KH, Bc], f32, kind="ExternalInput").ap()
    whh_d = {s: nc.dram_tensor(f"whht_{s}", [P, KH, H3], bf16,
                               kind="ExternalInput").ap() for s in "lr"}
    # packed per-side biases: cols [0:2K]=sigmoid(r,z), [2K:3K]=tanh, [3K:4K]=n_hh
    bias_d = {s: nc.dram_tensor(f"bias_{s}", [P, 4 * KH], f32,
                                kind="ExternalInput").ap() for s in "lr"}
    wout_d = nc.dram_tensor("woutt", [P, KH, V], f8, kind="ExternalInput").ap()
    bout_d = nc.dram_tensor("bout", [P, V], bf16, kind="ExternalInput").ap()
    out_d = nc.dram_tensor("out", [Bc, L, V], bf16, kind="ExternalOutput").ap()

    with tile.TileContext(nc) as tc, ExitStack() as ctx:
        constp = ctx.enter_context(tc.tile_pool(name="const", bufs=1))
        ht2p = ctx.enter_context(tc.tile_pool(name="ht2", bufs=1))
        ht2 = ht2p.tile([P, KH, TOK], f8)
        wvep = ctx.enter_context(tc.tile_pool(name="wout_early", bufs=1))
        bop = ctx.enter_context(tc.tile_pool(name="bout", bufs=1))

        bsig, btanh, bnhh = {}, {}, {}
        for s in "lr":
            bt = constp.tile([P, 4 * KH], f32, name=f"bias{s}")
            nc.sync.dma_start(out=bt, in_=bias_d[s])
            bsig[s] = bt[:, :2 * KH]
            btanh[s] = bt[:, 2 * KH:3 * KH]
            bnhh[s] = bt[:, 3 * KH:]

        def load_proj_consts():
            # issued AFTER the GRU weights so they don't head-block the
            # sync-engine DMA queue and delay the GRU start
            bout_sb = bop.tile([P, V], bf16)
            nc.sync.dma_start(out=bout_sb, in_=bout_d)
            wv = []
            for vg, (vs, vw) in enumerate(vgroups):
                wt = wvep.tile([P, KH, vw], f8, name=f"wv{vg}")
                nc.sync.dma_start(out=wt, in_=wout_d[:, :, vs:vs + vw])
                wv.append(wt)
            return bout_sb, wv

        # ---------------- GRU tree expansion ----------------
        NLL = max(1, min(P // Bc, L))  # leaves (nodes) per token chunk
        with tc.tile_pool(name="gwhh", bufs=1) as gwp, \
             tc.tile_pool(name="gh", bufs=1) as ghp, \
             tc.tile_pool(name="gact", bufs=2) as gap, \
             tc.tile_pool(name="gactd", bufs=1) as gdp, \
             tc.tile_pool(name="gpsum", bufs=2, space="PSUM") as gpp:
            # startup order: encoding (relu can start at once), then the GRU
            # weights the first level needs, then everything else
            enc_sb = gdp.tile([P, KH, Bc], f32, name="enc_stage")
            nc.sync.dma_start(out=enc_sb, in_=enc_d)
            whh = {}
            for s in "lr":
                w = gwp.tile([P, KH, H3], bf16, name=f"whh{s}")
                nc.sync.dma_start(out=w, in_=whh_d[s])
                whh[s] = w
            h_cur = ghp.tile([P, KH, Bc], bf16, name="h_l0")
            nc.scalar.activation(out=h_cur, in_=enc_sb, func=AF.Relu)

            bout_sb, wv = load_proj_consts()

            def emit_copies(h_last, si, t0, tt):
                """ht2 permute for the nodes finished by final-level tile
                (si, t0); node-grouped chunks: chunk tci holds nodes
                [NLL*tci, NLL*(tci+1)), interleaved p = b*NLL + (node % NLL)."""
                t_half = Bc << (depth - 1)
                lo = (si * t_half + t0) // Bc
                for node in range(lo, lo + tt // Bc):
                    base = (node // NLL) * P + node % NLL
                    nc.vector.tensor_copy(
                        out=ht2[:, :, base:base + (Bc - 1) * NLL + 1:NLL],
                        in_=h_last[:, :, node * Bc:(node + 1) * Bc])

            for lvl in range(depth):
                t = Bc << lvl
                h_nxt = ghp.tile([P, KH, 2 * t], bf16, name=f"h_l{lvl + 1}")
                for si, s in enumerate("lr"):
                    soff = si * t
                    for t0 in range(0, t, TTILE):
                        tt = min(TTILE, t - t0)
                        hs = h_cur[:, :, t0:t0 + tt]
                        r_sb = gap.tile([P, KH, TTILE], bf16, name="g_r")[:, :, :tt]
                        z_sb = gap.tile([P, KH, TTILE], bf16, name="g_z")[:, :, :tt]
                        n_sb = gap.tile([P, KH, TTILE], bf16, name="g_n")[:, :, :tt]
                        d_sb = gdp.tile([P, KH, TTILE], bf16, name="g_d")[:, :, :tt]
                        for gi in range(3):  # r, z, n
                            ps = gpp.tile([P, KH, TTILE], f32,
                                          name="g_ps")[:, :, :tt]
                            for gc in range(KH):
                                col = gi * H + gc * P
                                for k in range(KH):
                                    nc.tensor.matmul(
                                        ps[:, gc, :],
                                        lhsT=whh[s][:, k, col:col + P],
                                        rhs=hs[:, k, :],
                                        start=(k == 0), stop=(k == KH - 1))
                            if gi == 0:
                                for gc in range(KH):
                                    nc.scalar.activation(
                                        out=r_sb[:, gc, :], in_=ps[:, gc, :],
                                        func=AF.Sigmoid,
                                        bias=bsig[s][:, gc:gc + 1])
                            elif gi == 1:
                                for gc in range(KH):
                                    nc.scalar.activation(
                                        out=z_sb[:, gc, :], in_=ps[:, gc, :],
                                        func=AF.Sigmoid,
                                        bias=bsig[s][:, KH + gc:KH + gc + 1])
                            else:
                                for gc in range(KH):
                                    # n_pre = (gh_n + b_hh_n) * r
                                    nc.vector.scalar_tensor_tensor(
                                        out=n_sb[:, gc, :], in0=ps[:, gc, :],
                                        scalar=bnhh[s][:, gc:gc + 1],
                                        in1=r_sb[:, gc, :],
                                        op0=OP.add, op1=OP.mult)
                                for gc in range(KH):
                                    nc.scalar.activation(
                                        out=n_sb[:, gc, :], in_=n_sb[:, gc, :],
                                        func=AF.Tanh,
                                        bias=btanh[s][:, gc:gc + 1])
                        # h' = n + z * (h - n)
                        nc.vector.tensor_tensor(d_sb, hs, n_sb, OP.subtract)
                        nc.vector.tensor_tensor(d_sb, d_sb, z_sb, OP.mult)
                        nc.vector.tensor_tensor(
                            h_nxt[:, :, soff + t0:soff + t0 + tt],
                            d_sb, n_sb, OP.add)
                        # final level: stage finished nodes into ht2 (fp8)
                        # right away so the copies overlap remaining GRU work
                        if lvl == depth - 1:
                            emit_copies(h_nxt, si, t0, tt)
                h_cur = h_nxt

        # ---------------- output projection + log_softmax ----------------
        with tc.tile_pool(name="ypool", bufs=3) as yp, \
             tc.tile_pool(name="stat", bufs=3) as stp, \
             tc.tile_pool(name="escratch", bufs=2) as esp, \
             tc.tile_pool(name="ppsum", bufs=2, space="PSUM") as ppp:

            def out_dma(tci, pc, y, v0, v1):
                """Per-leaf stores; partitions are interleaved (p = b*NLL+ll)
                so each DMA's 32 source partitions stride across all 16 SBUF
                port groups and its rows spread over all 16 SDMA engines.
                Chunk tci holds NODES [NLL*tci, NLL*(tci+1)); the in-order
                leaf index is the bit-reversed node id."""
                nll = pc // Bc
                for ll in range(nll):
                    leaf = _bitrev(tci * nll + ll, depth)
                    nc.sync.dma_start(out=out_d[:, leaf, v0:v1],
                                      in_=y[ll:pc:nll, v0:v1])

            def tail_head(st):
                """c = ln(sum of exps); cs0 = c (for DVE subtract),
                cs1 = -c (ACT Identity bias)."""
                tci, pc, y, sums, cs = st
                nc.vector.tensor_reduce(out=cs[:, 0:1], in_=sums,
                                        axis=mybir.AxisListType.X, op=OP.add)
                nc.scalar.activation(out=cs[:, 0:1], in_=cs[:, 0:1],
                                     func=AF.Ln)
                nc.vector.tensor_scalar(out=cs[:, 1:2], in0=cs[:, 0:1],
                                        scalar1=-1.0, scalar2=None,
                                        op0=OP.mult)

            def tail_segment(st, a, b, eng):
                """out[a:b] = y[a:b] - c on the chosen engine."""
                tci, pc, y, sums, cs = st
                if eng == "A":
                    nc.scalar.activation(out=y[:, a:b], in_=y[:, a:b],
                                         func=AF.Identity, bias=cs[:, 1:2])
                elif eng == "G":
                    nc.gpsimd.tensor_scalar(out=y[:, a:b], in0=y[:, a:b],
                                            scalar1=cs[:, 0:1], scalar2=None,
                                            op0=OP.subtract)
                else:
                    nc.vector.tensor_scalar(out=y[:, a:b], in0=y[:, a:b],
                                            scalar1=cs[:, 0:1], scalar2=None,
                                            op0=OP.subtract)

            def seg(st, s):
                vs, vw = vgroups[s]
                tail_segment(st, vs, vs + vw, SEG_ENGINES[s])

            # two-deep tail pipeline: finals for chunk k run one vocab group
            # LATER than chunk k+1's drains so a burst of finals never delays
            # the drain that frees the next PSUM slot (which stalls the PE
            # long enough for HAM to re-throttle it)
            p0 = p1 = None
            for tci in range(NTC):
                pc = min(P, TOK - tci * P)  # tokens in this chunk
                y = yp.tile([P, V], bf16, name="y")[:pc]
                sums = stp.tile([P, NEG], f32, name="sums")[:pc]
                cs = stp.tile([P, 2], f32, name="cs")[:pc]
                ei = 0
                es = 0
                for vg, (vs, vw) in enumerate(vgroups):
                    ps = ppp.tile([P, VGW], f32, name="p_vg")[:pc, :vw]
                    for vt0 in range(0, vw, NBF):
                        w = min(NBF, vw - vt0)
                        pslice = ps[:, vt0:vt0 + w]
                        for kp in range(0, KH, 2):
                            nc.tensor.matmul(
                                pslice,
                                lhsT=ht2[:, kp:kp + 2, tci * P:tci * P + pc],
                                rhs=wv[vg][:, kp:kp + 2, vt0:vt0 + w],
                                start=(kp == 0), stop=(kp == KH - 2),
                                perf_mode=DR)
                    # PSUM drain + bias add -> y (bf16); single PSUM read
                    nc.vector.tensor_tensor(
                        y[:, vs:vs + vw], ps, bout_sb[:pc, vs:vs + vw], OP.add)
                    # exp reads y from SBUF (bf16); batched over vocab groups
                    if vg == exp_end_vg[ei]:
                        ew = vs + vw - es
                        esc = esp.tile([P, EGW], bf16, name="e_sc")[:pc, :ew]
                        nc.scalar.activation(out=esc, in_=y[:, es:es + ew],
                                             func=AF.Exp,
                                             accum_out=sums[:, ei:ei + 1])
                        es += ew
                        ei += 1
                    if vg == 0 and p0 is not None:
                        seg(p0, 4)
                        out_dma(p0[0], p0[1], p0[2], vgroups[4][0], V)
                    if p1 is not None:
                        if vg == 1:
                            tail_head(p1)
                            seg(p1, 0)
                        elif vg == 2:
                            seg(p1, 1)
                            out_dma(p1[0], p1[1], p1[2], 0, vgroups[2][0])
                        elif vg == 3:
                            seg(p1, 2)
                        elif vg == 4:
                            seg(p1, 3)
                            out_dma(p1[0], p1[1], p1[2],
                                    vgroups[2][0], vgroups[4][0])
                p0, p1 = p1, (tci, pc, y, sums, cs)
            # epilogue: finish the two still-pending tails
            seg(p0, 4)
            out_dma(p0[0], p0[1], p0[2], vgroups[4][0], V)
            tail_head(p1)
            nq = 8
            qs = [V * i // nq for i in range(nq + 1)]
            for i in range(nq):
                # all-DVE: ACT's last exp + Ln are on the critical path here
                tail_segment(p1, qs[i], qs[i + 1], "V")
                if i % 2 == 1 or i == nq - 1:
                    out_dma(p1[0], p1[1], p1[2], qs[i - (i % 2)], qs[i + 1])

    nc.compile()
    return nc


def _packed_bias(b_ih, b_hh, H, KH):
    """[P, 4*KH]: sigmoid biases (b_ih+b_hh for r,z), tanh bias (b_ih_n),
    and the pre-multiply n-gate bias (b_hh_n), per 128-row chunk."""
    P = 128
    sig = (b_ih + b_hh)[:2 * H].reshape(2 * KH, P).T
    tanh = b_ih[2 * H:].reshape(KH, P).T
    nhh = b_hh[2 * H:].reshape(KH, P).T
    return np.ascontiguousarray(np.concatenate([sig, tanh, nhh], axis=1))


def _get_compiled(Bc, H, V, depth):
    key = (Bc, H, V, depth)
    if key not in _COMPILE_CACHE:
        _COMPILE_CACHE[key] = _build(Bc, H, V, depth)
    return _COMPILE_CACHE[key]


def kernel(encoding, W_hh_l, b_ih_l, b_hh_l, W_hh_r, b_ih_r, b_hh_r,
           W_out, b_out, depth):
    global LAST_EXEC_NS, LAST_RESULTS
    encoding = np.asarray(encoding, np.float32)
    W_hh = {"l": np.asarray(W_hh_l, np.float32), "r": np.asarray(W_hh_r, np.float32)}
    b_ih = {"l": np.asarray(b_ih_l, np.float32), "r": np.asarray(b_ih_r, np.float32)}
    b_hh = {"l": np.asarray(b_hh_l, np.float32), "r": np.asarray(b_hh_r, np.float32)}
    W_out = np.asarray(W_out, np.float32)
    b_out = np.asarray(b_out, np.float32)
    depth = int(depth)

    B, H = encoding.shape
    V = W_out.shape[0]
    tok = (B // N_CORES) * (1 << depth) if B % N_CORES == 0 else 0
    if (depth < 1 or B % N_CORES or H % P or P % (B // N_CORES)
            or (tok % P != 0 and tok > P)):
        return _numpy_reference(encoding, W_hh["l"], b_ih["l"], b_hh["l"],
                                W_hh["r"], b_ih["r"], b_hh["r"],
                                W_out, b_out, depth).astype(np.float32)

    Bc = B // N_CORES
    KH = H // P
    bf16 = ml_dtypes.bfloat16
    f8 = ml_dtypes.float8_e4m3

    nc = _get_compiled(Bc, H, V, depth)

    # device layouts are [P(partition), KH, x]: H index = k*P + p -> axes (p, k)
    woutt = np.ascontiguousarray(
        W_out.T.astype(f8).reshape(KH, P, V).transpose(1, 0, 2))
    bout_b = np.ascontiguousarray(
        np.broadcast_to(b_out.astype(bf16)[None, :], (P, V)))
    shared = {"woutt": woutt, "bout": bout_b}
    for s in "lr":
        shared[f"whht_{s}"] = np.ascontiguousarray(
            W_hh[s].T.astype(bf16).reshape(KH, P, 3 * H).transpose(1, 0, 2))
        shared[f"bias_{s}"] = _packed_bias(b_ih[s], b_hh[s], H, KH)

    encT = encoding.T  # [H, B]
    in_maps = []
    for c in range(N_CORES):
        enc_c = np.ascontiguousarray(
            encT[:, c * Bc:(c + 1) * Bc].reshape(KH, P, Bc).transpose(1, 0, 2))
        in_maps.append({"enc_t": enc_c, **shared})

    from concourse import bass_utils
    kw = {}
    if TRACE:
        kw["tmpdir"] = os.environ.get("BASS_TRACE_DIR") or None
    res = bass_utils.run_bass_kernel_spmd(
        nc, in_maps, core_ids=list(range(N_CORES)), trace=TRACE, **kw)
    LAST_EXEC_NS = res.exec_time_ns
    LAST_RESULTS = res
    out = np.concatenate([r["out"] for r in res.results], axis=0)
    return np.ascontiguousarray(out.astype(np.float32))


# revision 28
# speedup vs baseline: 2.8080x; 2.8080x over previous
"""DecoderTreeRNN Trainium2 kernel.

Computes: h0 = relu(encoding); expand a depth-`depth` binary tree with two
zero-input GRU cells (left/right); project every leaf hidden state with W_out
and take log_softmax over the vocab.

Strategy: pure data parallel over 8 NeuronCores (batch sharded), GRU weights
and the output projection replicated.  On-core layout is transposed
([hidden-chunk on partitions, tokens on the free dim]) so all matmuls
contract over partitions and the softmax reduction runs along the free dim.

v2: the output projection runs in fp8 (DoubleRow matmuls, 2x tensor rate),
y and the stored output are bf16 (host upcasts to fp32; log-probs ~-9 so
bf16 keeps elementwise rel err ~2e-3, well under the 2e-2 gate), and the
log_softmax tail subtract is split DVE/ACT per segment.
"""

import os
import sys
from contextlib import ExitStack

import numpy as np

for _p in ("/opt/trn_rl_repo", "/root/.axon_site/_ro/trn_rl_repo"):
    if os.path.isdir(_p) and _p not in sys.path:
        sys.path.insert(0, _p)

import ml_dtypes

N_CORES = 8
P = 128
TTILE = 512  # token tile for GRU matmuls (max fp32 moving free dim)
NBF = 512  # fp32 elements per PSUM bank
VGW = 4 * NBF  # vocab group width (4 PSUM banks; 2 rotating slots)
EGW = VGW  # exp/accumulate granularity (per drain group: exp starts sooner)
# final-subtract engine per vocab segment within a chunk: V=DVE tensor_scalar
# (4x mode on bf16), A=ACT Identity+bias, G=GPSIMD tensor_scalar (ACT is the
# saturated engine in the projection; GPSIMD is otherwise idle)
SEG_ENGINES = ("V", "A", "V", "V", "V")
# output DMA issued after these vocab segments (batched; covers back to the
# previous flush point)
DMA_AFTER = (1, 3, 4)

# Set by test harness to capture a profile on the next kernel() call.
TRACE = False
LAST_EXEC_NS = None
LAST_RESULTS = None

_COMPILE_CACHE = {}


def _bitrev(x, bits):
    r = 0
    for _ in range(bits):
        r = (r << 1) | (x & 1)
        x >>= 1
    return r


def _numpy_reference(encoding, W_hh_l, b_ih_l, b_hh_l, W_hh_r, b_ih_r, b_hh_r,
                     W_out, b_out, depth):
    def gru(h, W, b_ih, b_hh):
        Hd = h.shape[-1]
        gh = h @ W.T + b_hh
        r = 1.0 / (1.0 + np.exp(-(b_ih[:Hd] + gh[..., :Hd])))
        z = 1.0 / (1.0 + np.exp(-(b_ih[Hd:2 * Hd] + gh[..., Hd:2 * Hd])))
        n = np.tanh(b_ih[2 * Hd:] + r * gh[..., 2 * Hd:])
        return (1.0 - z) * n + z * h

    h = np.maximum(encoding, 0.0)[:, None, :]
    for _ in range(depth):
        left = gru(h, W_hh_l, b_ih_l, b_hh_l)
        right = gru(h, W_hh_r, b_ih_r, b_hh_r)
        h = np.stack([left, right], axis=2).reshape(h.shape[0], -1, h.shape[-1])
    logits = h @ W_out.T + b_out
    m = logits.max(axis=-1, keepdims=True)
    e = np.exp(logits - m)
    return (logits - m) - np.log(e.sum(axis=-1, keepdims=True))


def _patch_act_tables(bacc, mybir):
    """Constrain the ACT table-set chooser so the GRU phase and the
    projection phase each stick to ONE set (2 loads total instead of 2
    per token chunk).  Only the chooser's view is filtered; the runtime
    tables are the real (full) sets, so execution is unchanged."""
    from concourse import hw_specs
    AF = mybir.ActivationFunctionType
    orig = hw_specs.get_activation_tables
    if getattr(bacc.get_activation_tables, "_treernn_patch", False):
        return
    keep = {
        "sigmoid_and_others": {AF.Sigmoid, AF.Tanh, AF.Relu},
        "natural_log_exp_and_others": {AF.Exp, AF.Ln, AF.Identity, AF.Copy},
    }
    controlled = set().union(*keep.values())

    def patched(arch):
        tabs = {k: set(v) for k, v in orig(arch).items()}
        for name, s in tabs.items():
            s -= controlled
            s |= keep.get(name, set())
        return tabs

    patched._treernn_patch = True
    bacc.get_activation_tables = patched


def _build(Bc, H, V, depth):
    """Build + compile the single-core SPMD program (identical on all cores)."""
    import concourse.bass as bass  # noqa: F401
    import concourse.tile as tile
    from concourse import bacc, mybir

    f32 = mybir.dt.float32
    bf16 = mybir.dt.bfloat16
    f8 = mybir.dt.float8e4
    AF = mybir.ActivationFunctionType
    OP = mybir.AluOpType
    DR = mybir.MatmulPerfMode.DoubleRow
    _patch_act_tables(bacc, mybir)

    KH = H // P
    H3 = 3 * H
    L = 1 << depth
    TOK = Bc * L
    NTC = (TOK + P - 1) // P

    def _chunks(width):
        out, pos = [], 0
        while pos < V:
            w = min(width, V - pos)
            out.append((pos, w))
            pos += w
        return out

    vgroups = _chunks(VGW)   # PSUM drain / final-subtract granularity
    NVG = len(vgroups)
    # exp groups end at these vgroup indices (vgroup-aligned, ~EGW wide)
    exp_end_vg = []
    acc = 0
    for vg, (vs, vw) in enumerate(vgroups):
        acc += vw
        if acc >= EGW or vg == NVG - 1:
            exp_end_vg.append(vg)
            acc = 0
    NEG = len(exp_end_vg)

    nc = bacc.Bacc("TRN2", target_bir_lowering=False, debug=False,
                   num_devices=N_CORES)

    enc_d = nc.dram_tensor("enc_t", [P, KH, Bc], f32, kind="ExternalInput").ap()
    whh_d = {s: nc.dram_tensor(f"whht_{s}", [P, KH, H3], bf16,
                               kind="ExternalInput").ap() for s in "lr"}
    # packed per-side biases: cols [0:2K]=sigmoid(r,z), [2K:3K]=tanh, [3K:4K]=n_hh
    bias_d = {s: nc.dram_tensor(f"bias_{s}", [P, 4 * KH], f32,
                                kind="ExternalInput").ap() for s in "lr"}
    wout_d = nc.dram_tensor("woutt", [P, KH, V], f8, kind="ExternalInput").ap()
    bout_d = nc.dram_tensor("bout", [P, V], bf16, kind="ExternalInput").ap()
    out_d = nc.dram_tensor("out", [Bc, L, V], bf16, kind="ExternalOutput").ap()

    with tile.TileContext(nc) as tc, ExitStack() as ctx:
        constp = ctx.enter_context(tc.tile_pool(name="const", bufs=1))
        ht2p = ctx.enter_context(tc.tile_pool(name="ht2", bufs=1))
        ht2 = ht2p.tile([P, KH, TOK], f8)
        wvep = ctx.enter_context(tc.tile_pool(name="wout_early", bufs=1))
        bop = ctx.enter_context(tc.tile_pool(name="bout", bufs=1))

        bsig, btanh, bnhh = {}, {}, {}
        for s in "lr":
            bt = constp.tile([P, 4 * KH], f32, name=f"bias{s}")
            nc.sync.dma_start(out=bt, in_=bias_d[s])
            bsig[s] = bt[:, :2 * KH]
            btanh[s] = bt[:, 2 * KH:3 * KH]
            bnhh[s] = bt[:, 3 * KH:]

        def load_proj_consts():
            # issued AFTER the GRU weights so they don't head-block the
            # sync-engine DMA queue and delay the GRU start
            bout_sb = bop.tile([P, V], bf16)
            nc.sync.dma_start(out=bout_sb, in_=bout_d)
            wv = []
            for vg, (vs, vw) in enumerate(vgroups):
                wt = wvep.tile([P, KH, vw], f8, name=f"wv{vg}")
                nc.sync.dma_start(out=wt, in_=wout_d[:, :, vs:vs + vw])
                wv.append(wt)
            return bout_sb, wv

        # ---------------- GRU tree expansion ----------------
        NLL = max(1, min(P // Bc, L))  # leaves (nodes) per token chunk
        with tc.tile_pool(name="gwhh", bufs=1) as gwp, \
             tc.tile_pool(name="gh", bufs=1) as ghp, \
             tc.tile_pool(name="gact", bufs=2) as gap, \
             tc.tile_pool(name="gactd", bufs=1) as gdp, \
             tc.tile_pool(name="gpsum", bufs=2, space="PSUM") as gpp:
            # startup order: encoding (relu can start at once), then the GRU
            # weights the first level needs, then everything else
            enc_sb = gdp.tile([P, KH, Bc], f32, name="enc_stage")
            nc.sync.dma_start(out=enc_sb, in_=enc_d)
            whh = {}
            for s in "lr":
                w = gwp.tile([P, KH, H3], bf16, name=f"whh{s}")
                nc.sync.dma_start(out=w, in_=whh_d[s])
                whh[s] = w
            h_cur = ghp.tile([P, KH, Bc], bf16, name="h_l0")
            nc.scalar.activation(out=h_cur, in_=enc_sb, func=AF.Relu)

            bout_sb, wv = load_proj_consts()

            def emit_copies(h_last, si, t0, tt):
                """ht2 permute for the nodes finished by final-level tile
                (si, t0); node-grouped chunks: chunk tci holds nodes
                [NLL*tci, NLL*(tci+1)), interleaved p = b*NLL + (node % NLL)."""
                t_half = Bc << (depth - 1)
                lo = (si * t_half + t0) // Bc
                for node in range(lo, lo + tt // Bc):
                    base = (node // NLL) * P + node % NLL
                    nc.vector.tensor_copy(
                        out=ht2[:, :, base:base + (Bc - 1) * NLL + 1:NLL],
                        in_=h_last[:, :, node * Bc:(node + 1) * Bc])

            for lvl in range(depth):
                t = Bc << lvl
                h_nxt = ghp.tile([P, KH, 2 * t], bf16, name=f"h_l{lvl + 1}")
                for si, s in enumerate("lr"):
                    soff = si * t
                    for t0 in range(0, t, TTILE):
                        tt = min(TTILE, t - t0)
                        hs = h_cur[:, :, t0:t0 + tt]
                        r_sb = gap.tile([P, KH, TTILE], bf16, name="g_r")[:, :, :tt]
                        z_sb = gap.tile([P, KH, TTILE], bf16, name="g_z")[:, :, :tt]
                        n_sb = gap.tile([P, KH, TTILE], bf16, name="g_n")[:, :, :tt]
                        d_sb = gdp.tile([P, KH, TTILE], bf16, name="g_d")[:, :, :tt]
                        for gi in range(3):  # r, z, n
                            ps = gpp.tile([P, KH, TTILE], f32,
                                          name="g_ps")[:, :, :tt]
                            for gc in range(KH):
                                col = gi * H + gc * P
                                for k in range(KH):
                                    nc.tensor.matmul(
                                        ps[:, gc, :],
                                        lhsT=whh[s][:, k, col:col + P],
                                        rhs=hs[:, k, :],
                                        start=(k == 0), stop=(k == KH - 1))
                            if gi == 0:
                                for gc in range(KH):
                                    nc.scalar.activation(
                                        out=r_sb[:, gc, :], in_=ps[:, gc, :],
                                        func=AF.Sigmoid,
                                        bias=bsig[s][:, gc:gc + 1])
                            elif gi == 1:
                                for gc in range(KH):
                                    nc.scalar.activation(
                                        out=z_sb[:, gc, :], in_=ps[:, gc, :],
                                        func=AF.Sigmoid,
                                        bias=bsig[s][:, KH + gc:KH + gc + 1])
                            else:
                                for gc in range(KH):
                                    # n_pre = (gh_n + b_hh_n) * r
                                    nc.vector.scalar_tensor_tensor(
                                        out=n_sb[:, gc, :], in0=ps[:, gc, :],
                                        scalar=bnhh[s][:, gc:gc + 1],
                                        in1=r_sb[:, gc, :],
                                        op0=OP.add, op1=OP.mult)
                                for gc in range(KH):
                                    nc.scalar.activation(
                                        out=n_sb[:, gc, :], in_=n_sb[:, gc, :],
                                        func=AF.Tanh,
                                        bias=btanh[s][:, gc:gc + 1])
                        # h' = n + z * (h - n)
                        nc.vector.tensor_tensor(d_sb, hs, n_sb, OP.subtract)
                        nc.vector.tensor_tensor(d_sb, d_sb, z_sb, OP.mult)
                        nc.vector.tensor_tensor(
                            h_nxt[:, :, soff + t0:soff + t0 + tt],
                            d_sb, n_sb, OP.add)
                        # final level: stage finished nodes into ht2 (fp8)
                        # right away so the copies overlap remaining GRU work
                        if lvl == depth - 1:
                            emit_copies(h_nxt, si, t0, tt)
                h_cur = h_nxt

        # ---------------- output projection + log_softmax ----------------
        with tc.tile_pool(name="ypool", bufs=3) as yp, \
             tc.tile_pool(name="stat", bufs=3) as stp, \
             tc.tile_pool(name="escratch", bufs=2) as esp, \
             tc.tile_pool(name="ppsum", bufs=2, space="PSUM") as ppp:

            def out_dma(tci, pc, y, v0, v1):
                """Per-leaf stores; partitions are interleaved (p = b*NLL+ll)
                so each DMA's 32 source partitions stride across all 16 SBUF
                port groups and its rows spread over all 16 SDMA engines.
                Chunk tci holds NODES [NLL*tci, NLL*(tci+1)); the in-order
                leaf index is the bit-reversed node id."""
                nll = pc // Bc
                for ll in range(nll):
                    leaf = _bitrev(tci * nll + ll, depth)
                    nc.sync.dma_start(out=out_d[:, leaf, v0:v1],
                                      in_=y[ll:pc:nll, v0:v1])

            def tail_head(st):
                """c = ln(sum of exps); cs0 = c (for DVE subtract),
                cs1 = -c (ACT Identity bias)."""
                tci, pc, y, sums, cs = st
                nc.vector.tensor_reduce(out=cs[:, 0:1], in_=sums,
                                        axis=mybir.AxisListType.X, op=OP.add)
                nc.scalar.activation(out=cs[:, 0:1], in_=cs[:, 0:1],
                                     func=AF.Ln)
                nc.vector.tensor_scalar(out=cs[:, 1:2], in0=cs[:, 0:1],
                                        scalar1=-1.0, scalar2=None,
                                        op0=OP.mult)

            def tail_segment(st, a, b, eng):
                """out[a:b] = y[a:b] - c on the chosen engine."""
                tci, pc, y, sums, cs = st
                if eng == "A":
                    nc.scalar.activation(out=y[:, a:b], in_=y[:, a:b],
                                         func=AF.Identity, bias=cs[:, 1:2])
                elif eng == "G":
                    nc.gpsimd.tensor_scalar(out=y[:, a:b], in0=y[:, a:b],
                                            scalar1=cs[:, 0:1], scalar2=None,
                                            op0=OP.subtract)
                else:
                    nc.vector.tensor_scalar(out=y[:, a:b], in0=y[:, a:b],
                                            scalar1=cs[:, 0:1], scalar2=None,
                                            op0=OP.subtract)

            def seg(st, s):
                vs, vw = vgroups[s]
                tail_segment(st, vs, vs + vw, SEG_ENGINES[s])

            # two-deep tail pipeline: finals for chunk k run one vocab group
            # LATER than chunk k+1's drains so a burst of finals never delays
            # the drain that frees the next PSUM slot (which stalls the PE
            # long enough for HAM to re-throttle it)
            p0 = p1 = None
            for tci in range(NTC):
                pc = min(P, TOK - tci * P)  # tokens in this chunk
                y = yp.tile([P, V], bf16, name="y")[:pc]
                sums = stp.tile([P, NEG], f32, name="sums")[:pc]
                cs = stp.tile([P, 2], f32, name="cs")[:pc]
                ei = 0
                es = 0
                for vg, (vs, vw) in enumerate(vgroups):
                    ps = ppp.tile([P, VGW], f32, name="p_vg")[:pc, :vw]
                    for vt0 in range(0, vw, NBF):
                        w = min(NBF, vw - vt0)
                        pslice = ps[:, vt0:vt0 + w]
                        for kp in range(0, KH, 2):
                            nc.tensor.matmul(
                                pslice,
                                lhsT=ht2[:, kp:kp + 2, tci * P:tci * P + pc],
                                rhs=wv[vg][:, kp:kp + 2, vt0:vt0 + w],
                                start=(kp == 0), stop=(kp == KH - 2),
                                perf_mode=DR)
                    # PSUM drain + bias add -> y (bf16); single PSUM read
                    nc.vector.tensor_tensor(
                        y[:, vs:vs + vw], ps, bout_sb[:pc, vs:vs + vw], OP.add)
                    # exp reads y from SBUF (bf16); batched over vocab groups
                    if vg == exp_end_vg[ei]:
                        ew = vs + vw - es
                        esc = esp.tile([P, EGW], bf16, name="e_sc")[:pc, :ew]
                        nc.scalar.activation(out=esc, in_=y[:, es:es + ew],
                                             func=AF.Exp,
                                             accum_out=sums[:, ei:ei + 1])
                        es += ew
                        ei += 1
                    if vg == 0 and p0 is not None:
                        seg(p0, 4)
                        out_dma(p0[0], p0[1], p0[2], vgroups[4][0], V)
                    if p1 is not None:
                        if vg == 1:
                            tail_head(p1)
                            seg(p1, 0)
                        elif vg == 2:
                            seg(p1, 1)
                            out_dma(p1[0], p1[1], p1[2], 0, vgroups[2][0])
                        elif vg == 3:
                            seg(p1, 2)
                        elif vg == 4:
                            seg(p1, 3)
                            out_dma(p1[0], p1[1], p1[2],
                                    vgroups[2][0], vgroups[4][0])
                p0, p1 = p1, (tci, pc, y, sums, cs)
            # epilogue: finish the two still-pending tails
            seg(p0, 4)
            out_dma(p0[0], p0[1], p0[2], vgroups[4][0], V)
            tail_head(p1)
            nq = 8
            qs = [V * i // nq for i in range(nq + 1)]
            for i in range(nq):
                # all-DVE: ACT's last exp + Ln are on the critical path here
                tail_segment(p1, qs[i], qs[i + 1], "V")
                if i % 2 == 1 or i == nq - 1:
                    out_dma(p1[0], p1[1], p1[2], qs[i - (i % 2)], qs[i + 1])

    nc.compile()
    return nc


def _packed_bias(b_ih, b_hh, H, KH):
    """[P, 4*KH]: sigmoid biases (b_ih+b_hh for r,z), tanh bias (b_ih_n),
    and the pre-multiply n-gate bias (b_hh_n), per 128-row chunk."""
    P = 128
    sig = (b_ih + b_hh)[:2 * H].reshape(2 * KH, P).T
    tanh = b_ih[2 * H:].reshape(KH, P).T
    nhh = b_hh[2 * H:].reshape(KH, P).T
    return np.ascontiguousarray(np.concatenate([sig, tanh, nhh], axis=1))


def _get_compiled(Bc, H, V, depth):
    key = (Bc, H, V, depth)
    if key not in _COMPILE_CACHE:
        _COMPILE_CACHE[key] = _build(Bc, H, V, depth)
    return _COMPILE_CACHE[key]


def kernel(encoding, W_hh_l, b_ih_l, b_hh_l, W_hh_r, b_ih_r, b_hh_r,
           W_out, b_out, depth):
    global LAST_EXEC_NS, LAST_RESULTS
    encoding = np.asarray(encoding, np.float32)
    W_hh = {"l": np.asarray(W_hh_l, np.float32), "r": np.asarray(W_hh_r, np.float32)}
    b_ih = {"l": np.asarray(b_ih_l, np.float32), "r": np.asarray(b_ih_r, np.float32)}
    b_hh = {"l": np.asarray(b_hh_l, np.float32), "r": np.asarray(b_hh_r, np.float32)}
    W_out = np.asarray(W_out, np.float32)
    b_out = np.asarray(b_out, np.float32)
    depth = int(depth)

    B, H = encoding.shape
    V = W_out.shape[0]
    tok = (B // N_CORES) * (1 << depth) if B % N_CORES == 0 else 0
    if (depth < 1 or B % N_CORES or H % P or P % (B // N_CORES)
            or (tok % P != 0 and tok > P)):
        return _numpy_reference(encoding, W_hh["l"], b_ih["l"], b_hh["l"],
                                W_hh["r"], b_ih["r"], b_hh["r"],
                                W_out, b_out, depth).astype(np.float32)

    Bc = B // N_CORES
    KH = H // P
    bf16 = ml_dtypes.bfloat16
    f8 = ml_dtypes.float8_e4m3

    nc = _get_compiled(Bc, H, V, depth)

    # device layouts are [P(partition), KH, x]: H index = k*P + p -> axes (p, k)
    woutt = np.ascontiguousarray(
        W_out.T.astype(f8).reshape(KH, P, V).transpose(1, 0, 2))
    bout_b = np.ascontiguousarray(
        np.broadcast_to(b_out.astype(bf16)[None, :], (P, V)))
    shared = {"woutt": woutt, "bout": bout_b}
    for s in "lr":
        shared[f"whht_{s}"] = np.ascontiguousarray(
            W_hh[s].T.astype(bf16).reshape(KH, P, 3 * H).transpose(1, 0, 2))
        shared[f"bias_{s}"] = _packed_bias(b_ih[s], b_hh[s], H, KH)

    encT = encoding.T  # [H, B]
    in_maps = []
    for c in range(N_CORES):
        enc_c = np.ascontiguousarray(
            encT[:, c * Bc:(c + 1) * Bc].reshape(KH, P, Bc).transpose(1, 0, 2))
        in_maps.append({"enc_t": enc_c, **shared})

    from concourse import bass_utils
    kw = {}
    if TRACE:
        kw["tmpdir"] = os.environ.get("BASS_TRACE_DIR") or None
    res = bass_utils.run_bass_kernel_spmd(
        nc, in_maps, core_ids=list(range(N_CORES)), trace=TRACE, **kw)
    LAST_EXEC_NS = res.exec_time_ns
    LAST_RESULTS = res
    out = np.concatenate([r["out"] for r in res.results], axis=0)
    return np.ascontiguousarray(out.astype(np.float32))


# revision 32
# speedup vs baseline: 2.8161x; 1.0029x over previous
"""DecoderTreeRNN Trainium2 kernel.

Computes: h0 = relu(encoding); expand a depth-`depth` binary tree with two
zero-input GRU cells (left/right); project every leaf hidden state with W_out
and take log_softmax over the vocab.

Strategy: pure data parallel over 8 NeuronCores (batch sharded), GRU weights
and the output projection replicated.  On-core layout is transposed
([hidden-chunk on partitions, tokens on the free dim]) so all matmuls
contract over partitions and the softmax reduction runs along the free dim.

v2: the output projection runs in fp8 (DoubleRow matmuls, 2x tensor rate),
y and the stored output are bf16 (host upcasts to fp32; log-probs ~-9 so
bf16 keeps elementwise rel err ~2e-3, well under the 2e-2 gate), and the
log_softmax tail subtract is split DVE/ACT per segment.
"""

import os
import sys
from contextlib import ExitStack

import numpy as np

for _p in ("/opt/trn_rl_repo", "/root/.axon_site/_ro/trn_rl_repo"):
    if os.path.isdir(_p) and _p not in sys.path:
        sys.path.insert(0, _p)

import ml_dtypes

N_CORES = 8
P = 128
TTILE = 512  # token tile for GRU matmuls (max fp32 moving free dim)
NBF = 512  # fp32 elements per PSUM bank
VGW = 4 * NBF  # vocab group width (4 PSUM banks; 2 rotating slots)
EGW = VGW  # exp/accumulate granularity (per drain group: exp starts sooner)
# final-subtract engine per vocab segment within a chunk: V=DVE tensor_scalar
# (4x mode on bf16), A=ACT Identity+bias, G=GPSIMD tensor_scalar (ACT is the
# saturated engine in the projection; GPSIMD is otherwise idle)
SEG_ENGINES = ("V", "A", "V", "A", "V")
# output DMA issued after these vocab segments (batched; covers back to the
# previous flush point)
DMA_AFTER = (1, 3, 4)

# Set by test harness to capture a profile on the next kernel() call.
TRACE = False
LAST_EXEC_NS = None
LAST_RESULTS = None

_COMPILE_CACHE = {}


def _bitrev(x, bits):
    r = 0
    for _ in range(bits):
        r = (r << 1) | (x & 1)
        x >>= 1
    return r


def _numpy_reference(encoding, W_hh_l, b_ih_l, b_hh_l, W_hh_r, b_ih_r, b_hh_r,
                     W_out, b_out, depth):
    def gru(h, W, b_ih, b_hh):
        Hd = h.shape[-1]
        gh = h @ W.T + b_hh
        r = 1.0 / (1.0 + np.exp(-(b_ih[:Hd] + gh[..., :Hd])))
        z = 1.0 / (1.0 + np.exp(-(b_ih[Hd:2 * Hd] + gh[..., Hd:2 * Hd])))
        n = np.tanh(b_ih[2 * Hd:] + r * gh[..., 2 * Hd:])
        return (1.0 - z) * n + z * h

    h = np.maximum(encoding, 0.0)[:, None, :]
    for _ in range(depth):
        left = gru(h, W_hh_l, b_ih_l, b_hh_l)
        right = gru(h, W_hh_r, b_ih_r, b_hh_r)
        h = np.stack([left, right], axis=2).reshape(h.shape[0], -1, h.shape[-1])
    logits = h @ W_out.T + b_out
    m = logits.max(axis=-1, keepdims=True)
    e = np.exp(logits - m)
    return (logits - m) - np.log(e.sum(axis=-1, keepdims=True))


def _patch_act_tables(bacc, mybir):
    """Constrain the ACT table-set chooser so the GRU phase and the
    projection phase each stick to ONE set (2 loads total instead of 2
    per token chunk).  Only the chooser's view is filtered; the runtime
    tables are the real (full) sets, so execution is unchanged."""
    from concourse import hw_specs
    AF = mybir.ActivationFunctionType
    orig = hw_specs.get_activation_tables
    if getattr(bacc.get_activation_tables, "_treernn_patch", False):
        return
    keep = {
        "sigmoid_and_others": {AF.Sigmoid, AF.Tanh, AF.Relu},
        "natural_log_exp_and_others": {AF.Exp, AF.Ln, AF.Identity, AF.Copy},
    }
    controlled = set().union(*keep.values())

    def patched(arch):
        tabs = {k: set(v) for k, v in orig(arch).items()}
        for name, s in tabs.items():
            s -= controlled
            s |= keep.get(name, set())
        return tabs

    patched._treernn_patch = True
    bacc.get_activation_tables = patched


def _build(Bc, H, V, depth):
    """Build + compile the single-core SPMD program (identical on all cores)."""
    import concourse.bass as bass  # noqa: F401
    import concourse.tile as tile
    from concourse import bacc, mybir

    f32 = mybir.dt.float32
    bf16 = mybir.dt.bfloat16
    f8 = mybir.dt.float8e4
    AF = mybir.ActivationFunctionType
    OP = mybir.AluOpType
    DR = mybir.MatmulPerfMode.DoubleRow
    _patch_act_tables(bacc, mybir)

    KH = H // P
    H3 = 3 * H
    L = 1 << depth
    TOK = Bc * L
    NTC = (TOK + P - 1) // P

    def _chunks(width):
        out, pos = [], 0
        while pos < V:
            w = min(width, V - pos)
            out.append((pos, w))
            pos += w
        return out

    vgroups = _chunks(VGW)   # PSUM drain / final-subtract granularity
    NVG = len(vgroups)
    # exp groups end at these vgroup indices (vgroup-aligned, ~EGW wide)
    exp_end_vg = []
    acc = 0
    for vg, (vs, vw) in enumerate(vgroups):
        acc += vw
        if acc >= EGW or vg == NVG - 1:
            exp_end_vg.append(vg)
            acc = 0
    NEG = len(exp_end_vg)

    nc = bacc.Bacc("TRN2", target_bir_lowering=False, debug=False,
                   num_devices=N_CORES)

    enc_d = nc.dram_tensor("enc_t", [P, KH, Bc], f32, kind="ExternalInput").ap()
    whh_d = {s: nc.dram_tensor(f"whht_{s}", [P, KH, H3], bf16,
                               kind="ExternalInput").ap() for s in "lr"}
    # packed per-side biases: cols [0:2K]=sigmoid(r,z), [2K:3K]=tanh, [3K:4K]=n_hh
    bias_d = {s: nc.dram_tensor(f"bias_{s}", [P, 4 * KH], f32,
                                kind="ExternalInput").ap() for s in "lr"}
    wout_d = nc.dram_tensor("woutt", [P, KH, V], f8, kind="ExternalInput").ap()
    bout_d = nc.dram_tensor("bout", [P, V], bf16, kind="ExternalInput").ap()
    out_d = nc.dram_tensor("out", [Bc, L, V], bf16, kind="ExternalOutput").ap()

    with tile.TileContext(nc) as tc, ExitStack() as ctx:
        constp = ctx.enter_context(tc.tile_pool(name="const", bufs=1))
        ht2p = ctx.enter_context(tc.tile_pool(name="ht2", bufs=1))
        ht2 = ht2p.tile([P, KH, TOK], f8)
        wvep = ctx.enter_context(tc.tile_pool(name="wout_early", bufs=1))
        bop = ctx.enter_context(tc.tile_pool(name="bout", bufs=1))

        bsig, btanh, bnhh = {}, {}, {}
        for s in "lr":
            bt = constp.tile([P, 4 * KH], f32, name=f"bias{s}")
            nc.sync.dma_start(out=bt, in_=bias_d[s])
            bsig[s] = bt[:, :2 * KH]
            btanh[s] = bt[:, 2 * KH:3 * KH]
            bnhh[s] = bt[:, 3 * KH:]

        def load_proj_consts():
            # issued AFTER the GRU weights so they don't head-block the
            # sync-engine DMA queue and delay the GRU start
            bout_sb = bop.tile([P, V], bf16)
            nc.sync.dma_start(out=bout_sb, in_=bout_d)
            wv = []
            for vg, (vs, vw) in enumerate(vgroups):
                wt = wvep.tile([P, KH, vw], f8, name=f"wv{vg}")
                nc.sync.dma_start(out=wt, in_=wout_d[:, :, vs:vs + vw])
                wv.append(wt)
            return bout_sb, wv

        # ---------------- GRU tree expansion ----------------
        NLL = max(1, min(P // Bc, L))  # leaves (nodes) per token chunk
        with tc.tile_pool(name="gwhh", bufs=1) as gwp, \
             tc.tile_pool(name="gh", bufs=1) as ghp, \
             tc.tile_pool(name="gact", bufs=2) as gap, \
             tc.tile_pool(name="gactd", bufs=1) as gdp, \
             tc.tile_pool(name="gpsum", bufs=2, space="PSUM") as gpp:
            # startup order: encoding (relu can start at once), then the GRU
            # weights the first level needs, then everything else
            enc_sb = gdp.tile([P, KH, Bc], f32, name="enc_stage")
            nc.sync.dma_start(out=enc_sb, in_=enc_d)
            whh = {}
            for s in "lr":
                w = gwp.tile([P, KH, H3], bf16, name=f"whh{s}")
                # per-gate-block loads: level 0's first matmul only waits for
                # the r-gate third of whh_l, not the full 1.5 MB tensor
                for g in range(3):
                    nc.sync.dma_start(out=w[:, :, g * H:(g + 1) * H],
                                      in_=whh_d[s][:, :, g * H:(g + 1) * H])
                whh[s] = w
            h_cur = ghp.tile([P, KH, Bc], bf16, name="h_l0")
            nc.scalar.activation(out=h_cur, in_=enc_sb, func=AF.Relu)

            bout_sb, wv = load_proj_consts()

            def emit_copies(h_last, si, t0, tt):
                """ht2 permute for the nodes finished by final-level tile
                (si, t0); node-grouped chunks: chunk tci holds nodes
                [NLL*tci, NLL*(tci+1)), interleaved p = b*NLL + (node % NLL)."""
                t_half = Bc << (depth - 1)
                lo = (si * t_half + t0) // Bc
                for node in range(lo, lo + tt // Bc):
                    base = (node // NLL) * P + node % NLL
                    nc.vector.tensor_copy(
                        out=ht2[:, :, base:base + (Bc - 1) * NLL + 1:NLL],
                        in_=h_last[:, :, node * Bc:(node + 1) * Bc])

            for lvl in range(depth):
                t = Bc << lvl
                h_nxt = ghp.tile([P, KH, 2 * t], bf16, name=f"h_l{lvl + 1}")
                for si, s in enumerate("lr"):
                    soff = si * t
                    for t0 in range(0, t, TTILE):
                        tt = min(TTILE, t - t0)
                        hs = h_cur[:, :, t0:t0 + tt]
                        r_sb = gap.tile([P, KH, TTILE], bf16, name="g_r")[:, :, :tt]
                        z_sb = gap.tile([P, KH, TTILE], bf16, name="g_z")[:, :, :tt]
                        n_sb = gap.tile([P, KH, TTILE], bf16, name="g_n")[:, :, :tt]
                        d_sb = gdp.tile([P, KH, TTILE], bf16, name="g_d")[:, :, :tt]
                        for gi in range(3):  # r, z, n
                            ps = gpp.tile([P, KH, TTILE], f32,
                                          name="g_ps")[:, :, :tt]
                            for gc in range(KH):
                                col = gi * H + gc * P
                                for k in range(KH):
                                    nc.tensor.matmul(
                                        ps[:, gc, :],
                                        lhsT=whh[s][:, k, col:col + P],
                                        rhs=hs[:, k, :],
                                        start=(k == 0), stop=(k == KH - 1))
                            if gi == 0:
                                for gc in range(KH):
                                    nc.scalar.activation(
                                        out=r_sb[:, gc, :], in_=ps[:, gc, :],
                                        func=AF.Sigmoid,
                                        bias=bsig[s][:, gc:gc + 1])
                            elif gi == 1:
                                for gc in range(KH):
                                    nc.scalar.activation(
                                        out=z_sb[:, gc, :], in_=ps[:, gc, :],
                                        func=AF.Sigmoid,
                                        bias=bsig[s][:, KH + gc:KH + gc + 1])
                            else:
                                for gc in range(KH):
                                    # n_pre = (gh_n + b_hh_n) * r
                                    nc.vector.scalar_tensor_tensor(
                                        out=n_sb[:, gc, :], in0=ps[:, gc, :],
                                        scalar=bnhh[s][:, gc:gc + 1],
                                        in1=r_sb[:, gc, :],
                                        op0=OP.add, op1=OP.mult)
                                for gc in range(KH):
                                    nc.scalar.activation(
                                        out=n_sb[:, gc, :], in_=n_sb[:, gc, :],
                                        func=AF.Tanh,
                                        bias=btanh[s][:, gc:gc + 1])
                        # h' = n + z * (h - n)
                        nc.vector.tensor_tensor(d_sb, hs, n_sb, OP.subtract)
                        nc.vector.tensor_tensor(d_sb, d_sb, z_sb, OP.mult)
                        nc.vector.tensor_tensor(
                            h_nxt[:, :, soff + t0:soff + t0 + tt],
                            d_sb, n_sb, OP.add)
                        # final level: stage finished nodes into ht2 (fp8)
                        # right away so the copies overlap remaining GRU work
                        if lvl == depth - 1:
                            emit_copies(h_nxt, si, t0, tt)
                h_cur = h_nxt

        # ---------------- output projection + log_softmax ----------------
        with tc.tile_pool(name="ypool", bufs=3) as yp, \
             tc.tile_pool(name="stat", bufs=3) as stp, \
             tc.tile_pool(name="escratch", bufs=2) as esp, \
             tc.tile_pool(name="ppsum", bufs=2, space="PSUM") as ppp:

            def out_dma(tci, pc, y, v0, v1):
                """Per-leaf stores; partitions are interleaved (p = b*NLL+ll)
                so each DMA's 32 source partitions stride across all 16 SBUF
                port groups and its rows spread over all 16 SDMA engines.
                Chunk tci holds NODES [NLL*tci, NLL*(tci+1)); the in-order
                leaf index is the bit-reversed node id."""
                nll = pc // Bc
                for ll in range(nll):
                    leaf = _bitrev(tci * nll + ll, depth)
                    nc.sync.dma_start(out=out_d[:, leaf, v0:v1],
                                      in_=y[ll:pc:nll, v0:v1])

            def tail_head(st):
                """c = ln(sum of exps); cs0 = c (for DVE subtract),
                cs1 = -c (ACT Identity bias)."""
                tci, pc, y, sums, cs = st
                nc.vector.tensor_reduce(out=cs[:, 0:1], in_=sums,
                                        axis=mybir.AxisListType.X, op=OP.add)
                nc.scalar.activation(out=cs[:, 0:1], in_=cs[:, 0:1],
                                     func=AF.Ln)
                nc.vector.tensor_scalar(out=cs[:, 1:2], in0=cs[:, 0:1],
                                        scalar1=-1.0, scalar2=None,
                                        op0=OP.mult)

            def tail_segment(st, a, b, eng):
                """out[a:b] = y[a:b] - c on the chosen engine."""
                tci, pc, y, sums, cs = st
                if eng == "A":
                    nc.scalar.activation(out=y[:, a:b], in_=y[:, a:b],
                                         func=AF.Identity, bias=cs[:, 1:2])
                elif eng == "G":
                    nc.gpsimd.tensor_scalar(out=y[:, a:b], in0=y[:, a:b],
                                            scalar1=cs[:, 0:1], scalar2=None,
                                            op0=OP.subtract)
                else:
                    nc.vector.tensor_scalar(out=y[:, a:b], in0=y[:, a:b],
                                            scalar1=cs[:, 0:1], scalar2=None,
                                            op0=OP.subtract)

            def seg(st, s):
                vs, vw = vgroups[s]
                tail_segment(st, vs, vs + vw, SEG_ENGINES[s])

            # two-deep tail pipeline: finals for chunk k run one vocab group
            # LATER than chunk k+1's drains so a burst of finals never delays
            # the drain that frees the next PSUM slot (which stalls the PE
            # long enough for HAM to re-throttle it)
            p0 = p1 = None
            for tci in range(NTC):
                pc = min(P, TOK - tci * P)  # tokens in this chunk
                y = yp.tile([P, V], bf16, name="y")[:pc]
                sums = stp.tile([P, NEG], f32, name="sums")[:pc]
                cs = stp.tile([P, 2], f32, name="cs")[:pc]
                ei = 0
                es = 0
                for vg, (vs, vw) in enumerate(vgroups):
                    ps = ppp.tile([P, VGW], f32, name="p_vg")[:pc, :vw]
                    for vt0 in range(0, vw, NBF):
                        w = min(NBF, vw - vt0)
                        pslice = ps[:, vt0:vt0 + w]
                        for kp in range(0, KH, 2):
                            nc.tensor.matmul(
                                pslice,
                                lhsT=ht2[:, kp:kp + 2, tci * P:tci * P + pc],
                                rhs=wv[vg][:, kp:kp + 2, vt0:vt0 + w],
                                start=(kp == 0), stop=(kp == KH - 2),
                                perf_mode=DR)
                    # PSUM drain + bias add -> y (bf16); single PSUM read
                    nc.vector.tensor_tensor(
                        y[:, vs:vs + vw], ps, bout_sb[:pc, vs:vs + vw], OP.add)
                    # exp reads y from SBUF (bf16); batched over vocab groups
                    if vg == exp_end_vg[ei]:
                        ew = vs + vw - es
                        esc = esp.tile([P, EGW], bf16, name="e_sc")[:pc, :ew]
                        nc.scalar.activation(out=esc, in_=y[:, es:es + ew],
                                             func=AF.Exp,
                                             accum_out=sums[:, ei:ei + 1])
                        es += ew
                        ei += 1
                    if vg == 0 and p0 is not None:
                        seg(p0, 4)
                        out_dma(p0[0], p0[1], p0[2], vgroups[4][0], V)
                    if p1 is not None:
                        if vg == 1:
                            tail_head(p1)
                            seg(p1, 0)
                        elif vg == 2:
                            seg(p1, 1)
                            out_dma(p1[0], p1[1], p1[2], 0, vgroups[2][0])
                        elif vg == 3:
                            seg(p1, 2)
                        elif vg == 4:
                            seg(p1, 3)
                            out_dma(p1[0], p1[1], p1[2],
                                    vgroups[2][0], vgroups[4][0])
                p0, p1 = p1, (tci, pc, y, sums, cs)
            # epilogue: finish the two still-pending tails
            seg(p0, 4)
            out_dma(p0[0], p0[1], p0[2], vgroups[4][0], V)
            tail_head(p1)
            nq = 8
            qs = [V * i // nq for i in range(nq + 1)]
            for i in range(nq):
                tail_segment(p1, qs[i], qs[i + 1],
                             "V" if i % 2 == 0 else "A")
                # kernel-end drain: store each slice as soon as it's ready
                out_dma(p1[0], p1[1], p1[2], qs[i], qs[i + 1])

    nc.compile()
    return nc


def _packed_bias(b_ih, b_hh, H, KH):
    """[P, 4*KH]: sigmoid biases (b_ih+b_hh for r,z), tanh bias (b_ih_n),
    and the pre-multiply n-gate bias (b_hh_n), per 128-row chunk."""
    P = 128
    sig = (b_ih + b_hh)[:2 * H].reshape(2 * KH, P).T
    tanh = b_ih[2 * H:].reshape(KH, P).T
    nhh = b_hh[2 * H:].reshape(KH, P).T
    return np.ascontiguousarray(np.concatenate([sig, tanh, nhh], axis=1))


def _get_compiled(Bc, H, V, depth):
    key = (Bc, H, V, depth)
    if key not in _COMPILE_CACHE:
        _COMPILE_CACHE[key] = _build(Bc, H, V, depth)
    return _COMPILE_CACHE[key]


def kernel(encoding, W_hh_l, b_ih_l, b_hh_l, W_hh_r, b_ih_r, b_hh_r,
           W_out, b_out, depth):
    global LAST_EXEC_NS, LAST_RESULTS
    encoding = np.asarray(encoding, np.float32)
    W_hh = {"l": np.asarray(W_hh_l, np.float32), "r": np.asarray(W_hh_r, np.float32)}
    b_ih = {"l": np.asarray(b_ih_l, np.float32), "r": np.asarray(b_ih_r, np.float32)}
    b_hh = {"l": np.asarray(b_hh_l, np.float32), "r": np.asarray(b_hh_r, np.float32)}
    W_out = np.asarray(W_out, np.float32)
    b_out = np.asarray(b_out, np.float32)
    depth = int(depth)

    B, H = encoding.shape
    V = W_out.shape[0]
    tok = (B // N_CORES) * (1 << depth) if B % N_CORES == 0 else 0
    if (depth < 1 or B % N_CORES or H % P or P % (B // N_CORES)
            or (tok % P != 0 and tok > P)):
        return _numpy_reference(encoding, W_hh["l"], b_ih["l"], b_hh["l"],
                                W_hh["r"], b_ih["r"], b_hh["r"],
                                W_out, b_out, depth).astype(np.float32)

    Bc = B // N_CORES
    KH = H // P
    bf16 = ml_dtypes.bfloat16
    f8 = ml_dtypes.float8_e4m3

    nc = _get_compiled(Bc, H, V, depth)

    # device layouts are [P(partition), KH, x]: H index = k*P + p -> axes (p, k)
    woutt = np.ascontiguousarray(
        W_out.T.astype(f8).reshape(KH, P, V).transpose(1, 0, 2))
    bout_b = np.ascontiguousarray(
        np.broadcast_to(b_out.astype(bf16)[None, :], (P, V)))
    shared = {"woutt": woutt, "bout": bout_b}
    for s in "lr":
        shared[f"whht_{s}"] = np.ascontiguousarray(
            W_hh[s].T.astype(bf16).reshape(KH, P, 3 * H).transpose(1, 0, 2))
        shared[f"bias_{s}"] = _packed_bias(b_ih[s], b_hh[s], H, KH)

    encT = encoding.T  # [H, B]
    in_maps = []
    for c in range(N_CORES):
        enc_c = np.ascontiguousarray(
            encT[:, c * Bc:(c + 1) * Bc].reshape(KH, P, Bc).transpose(1, 0, 2))
        in_maps.append({"enc_t": enc_c, **shared})

    from concourse import bass_utils
    kw = {}
    if TRACE:
        kw["tmpdir"] = os.environ.get("BASS_TRACE_DIR") or None
    res = bass_utils.run_bass_kernel_spmd(
        nc, in_maps, core_ids=list(range(N_CORES)), trace=TRACE, **kw)
    LAST_EXEC_NS = res.exec_time_ns
    LAST_RESULTS = res
    out = np.concatenate([r["out"] for r in res.results], axis=0)
    return np.ascontiguousarray(out.astype(np.float32))
